# revision 26
# baseline (speedup 1.0000x reference)
"""Trainium2 Bass kernel for nn_BSLSegmenterV0 (histogram-binning weighted CE).

Math (target is exactly one-hot over the class axis C):
    cf[c]  = sum_n target[n, c]                      (global class histogram)
    S1     = sum_{n,c} target[n,c] * pred[n,c]
    S2     = sum_c cf[c] * ln(cf[c])
    S3     = sum_n ln( sum_c exp(pred[n,c]) * cf[c] )
    out    = -(S1 + S2 - S3) / N

Sharding: batch-parallel over 8 NeuronCores (one batch image each). The only
cross-core dependency is the 126-float cf partial histogram (AllGather +
on-chip fold); S1/S3 partials are returned per core and combined on the host.

Per-core dataflow (classes on partitions, pixels on the free axis; the host
pre-arranges each shard to [n_chunks*C, tile_f] chunk-major so every device
DMA is a contiguous 2-D block):
  pass A (streamed, DMA-bound): exp(pred) kept resident in SBUF as bf16;
      cf partials via ScalarE activation accum_out; S1 via VectorE mul+reduce.
  cf -> AllGather -> global cf -> block-diag bf16 W. This chain runs entirely
      on ScalarE + DMA so it never queues behind VectorE's pass-A backlog.
  pass B (from resident SBUF): per full tile, 4 col-tiled PE matmuls
      A = W^T @ exp(pred) fill one PSUM bank [128, 512] (rows 32j..32j+5 are
      real groups, rows 32j+6..32j+31 are forced to 1.0 via a ones-row in the
      moving tile and ones-columns in W, so ln() of the whole bank is safe);
      one ScalarE Ln activation with accum_out per tile yields sum ln(A).
"""

import os
import sys

for _p in ("/opt/trn_rl_repo", "/root/.axon_site/_ro/trn_rl_repo"):
    if os.path.isdir(_p) and _p not in sys.path:
        sys.path.append(_p)

import ml_dtypes
import numpy as np

import concourse.bacc as bacc
import concourse.bass as bass
import concourse.mybir as mybir
import concourse.tile as tile
from concourse.bass_utils import run_bass_kernel_spmd

F32 = mybir.dt.float32
BF16 = mybir.dt.bfloat16
Act = mybir.ActivationFunctionType

# full-problem config
B, C, H, W = 8, 21, 512, 512
N_CORES = 8
NPIX = H * W              # pixels per core (one batch image per core)
TILE_F = 2048             # pixels per chunk (free-dim of a stream tile)
MM_F = 512                # matmul moving free dim (one PSUM bank of fp32)


def build(n_cores=N_CORES, c=C, npix=NPIX, tile_f=TILE_F, mm_f=MM_F):
    """Build the SPMD Bass program. Returns (nc, meta)."""
    assert npix % tile_f == 0
    n_chunks = npix // tile_f
    g_full = 128 // c                      # class-groups stacked on partitions
    n_full = n_chunks // g_full            # full tiles
    rem_g = n_chunks % g_full              # groups in the remainder tile
    tiles = [g_full] * n_full + ([rem_g] if rem_g else [])
    nt = len(tiles)
    mm_per_tile = tile_f // mm_f
    assert mm_per_tile <= 4, "col-tiling uses one PSUM bank of 4 x 32 rows"
    pmax = g_full * c                      # 126

    nc = bacc.Bacc("TRN2", target_bir_lowering=False, debug=False,
                   num_devices=n_cores)

    # host pre-arranged layout: row (chunk*c + class), contiguous rows
    pred_d = nc.dram_tensor("pred", [n_chunks * c, tile_f], F32,
                            kind="ExternalInput").ap()
    tgt_d = nc.dram_tensor("tgt", [n_chunks * c, tile_f], F32,
                           kind="ExternalInput").ap()
    s1c_d = nc.dram_tensor("s1c", [pmax, nt], F32, kind="ExternalOutput").ap()
    bank_rows = 32 * mm_per_tile
    s3c_d = nc.dram_tensor("s3c", [bank_rows, max(n_full, 1)], F32,
                           kind="ExternalOutput").ap()
    s3r_d = nc.dram_tensor("s3r", [max(rem_g, 1), mm_per_tile], F32,
                           kind="ExternalOutput").ap()
    cfg_d = nc.dram_tensor("cfg", [c, 1], F32, kind="ExternalOutput").ap()

    cc_space = "Shared" if n_cores > 4 else "Local"
    cc_in = nc.dram_tensor("cc_in", [pmax], F32)
    cc_out = nc.dram_tensor("cc_out", [n_cores * pmax], F32,
                            addr_space=cc_space)
    dum_in = nc.dram_tensor("dum_in", [32], F32)
    dum_out = nc.dram_tensor("dum_out", [n_cores * 32], F32,
                             addr_space=cc_space)
    ones_d = nc.inline_tensor(
        np.ones((1, tile_f), dtype=ml_dtypes.bfloat16), name="ones_bf16")

    with tile.TileContext(nc) as tc:
        with (
            tc.tile_pool(name="tstreams", bufs=5) as tstreams,
            tc.tile_pool(name="pstreams", bufs=4) as pstreams,
            tc.tile_pool(name="scratch", bufs=2) as scratch,
            tc.tile_pool(name="resident", bufs=1) as resident,
            tc.tile_pool(name="stats", bufs=1) as stats,
            tc.tile_pool(name="psum", bufs=6, space="PSUM") as psum,
        ):
            cf_cols = stats.tile([pmax, nt], F32, tag="cf_cols")
            s1_cols = stats.tile([pmax, nt], F32, tag="s1_cols")
            s3_cols = stats.tile([bank_rows, max(n_full, 1)], F32, tag="s3_cols")
            s3_rem = stats.tile([max(rem_g, 1), mm_per_tile], F32, tag="s3_rem")
            # W: [127, 32] bf16; cols 0..g_full-1 block-diag cf, row 126 has
            # 1.0 in cols g_full..31 (pairs with the ones-row of moving tiles
            # so col-tiled PSUM pad rows become ln(1.0))
            w_sb = stats.tile([pmax + 1, 32], BF16, tag="w_sb")
            nc.scalar.memzero(w_sb[:])
            # warm up the ncfw collective path early (overlaps pass A)
            dum_sb = stats.tile([32, 1], F32, tag="dum_sb")
            nc.vector.memset(dum_sb[:], 0.0)
            nc.sync.dma_start(dum_in[:], dum_sb[:, 0])
            nc.gpsimd.collective_compute(
                "AllGather", mybir.AluOpType.bypass,
                replica_groups=[list(range(n_cores))],
                ins=[dum_in[:]], outs=[dum_out[:]])
            nc.sync.dma_start(w_sb[pmax:pmax + 1, g_full:32],
                              ones_d[0:1, 0:32 - g_full])
            if rem_g:
                # remainder cf column fills only rem_g*c rows; pre-zero the
                # whole column (engine ops need 32-aligned start partitions,
                # so we can't zero just the tail)
                nc.scalar.memzero(cf_cols[:, n_full:n_full + 1])

            # ---- pass A: stream target+pred, build resident exp(pred) ----
            # tgt DMAs are issued LEAD tiles ahead of pred DMAs so the class
            # histogram (collective input) completes well before pred streaming
            # does, hiding the AllGather under the pass-A tail.
            LEAD = 4
            t_tiles = []

            def issue_tgt(t, g):
                p = g * c
                r0 = t * g_full * c
                t_tile = tstreams.tile([p, tile_f], F32, tag="t_stream")
                nc.sync.dma_start(t_tile[:], tgt_d[r0:r0 + p, :])
                t_tiles.append(t_tile)

            for t in range(min(LEAD, nt)):
                issue_tgt(t, tiles[t])
            exp_res = []
            for t, g in enumerate(tiles):
                p = g * c
                r0 = t * g_full * c
                p_tile = pstreams.tile([p, tile_f], F32, tag="p_stream")
                nc.sync.dma_start(p_tile[:], pred_d[r0:r0 + p, :])
                if t + LEAD < nt:
                    issue_tgt(t + LEAD, tiles[t + LEAD])
                t_tile = t_tiles[t]

                full = g == g_full
                e_tile = resident.tile([p + (1 if full else 0), tile_f], BF16,
                                       tag=f"exp{t}")
                exp_res.append(e_tile)
                if full:  # ones-row pairs with W's ones-columns in pass B
                    nc.gpsimd.dma_start(e_tile[p:p + 1, :], ones_d[0:1, :])
                nc.scalar.activation(e_tile[0:p, :], p_tile[:], Act.Exp)
                # cf partial: ScalarE identity with accumulate output
                a_scr = scratch.tile([p, tile_f], mybir.dt.float8e4, tag="a_scr")
                nc.scalar.activation(a_scr[:], t_tile[:], Act.Identity,
                                     accum_out=cf_cols[0:p, t:t + 1])
                # S1 partial: (tgt*pred) then free-axis reduce on VectorE
                v_scr = scratch.tile([p, tile_f], BF16, tag="v_scr")
                nc.vector.tensor_mul(v_scr[:], t_tile[:], p_tile[:])
                nc.vector.tensor_reduce(s1_cols[0:p, t:t + 1], v_scr[:],
                                        axis=mybir.AxisListType.X,
                                        op=mybir.AluOpType.add)

            # ---- cf: ScalarE pre-fold -> AllGather [126] -> fold -> W ----
            cf_part = stats.tile([pmax, 1], F32, tag="cf_part")
            f_scr = stats.tile([pmax, nt], BF16, tag="f_scr")
            nc.scalar.activation(f_scr[:], cf_cols[:], Act.Identity,
                                 accum_out=cf_part[:])
            nc.sync.dma_start(cc_in[:], cf_part[:, 0])
            nc.gpsimd.collective_compute(
                "AllGather", mybir.AluOpType.bypass,
                replica_groups=[list(range(n_cores))],
                ins=[cc_in[:]], outs=[cc_out[:]])
            # dram element (r, j, ch) -> sbuf [ch, (r j)]
            ncols = n_cores * g_full
            cf_all = stats.tile([c, ncols], F32, tag="cf_all")
            nc.sync.dma_start(
                cf_all[:].rearrange("ch (r j) -> ch r j", r=n_cores),
                cc_out.rearrange("(r j ch) -> ch r j", ch=c, j=g_full))
            cf_g = stats.tile([c, 1], F32, tag="cf_g")
            g_scr = stats.tile([c, ncols], BF16, tag="g_scr")
            nc.scalar.activation(g_scr[:], cf_all[:], Act.Identity,
                                 accum_out=cf_g[:])
            nc.sync.dma_start(cfg_d[:], cf_g[:])
            cf_gb = stats.tile([c, 1], BF16, tag="cf_gb")
            nc.scalar.activation(cf_gb[:], cf_g[:], Act.Copy)
            for j in range(g_full):
                nc.sync.dma_start(w_sb[j * c:(j + 1) * c, j:j + 1], cf_gb[:])

            # ---- pass B: A = W^T @ exp(pred); S3 += sum ln(A) ----
            for t, g in enumerate(tiles):
                p = g * c
                if g == g_full:
                    ps = psum.tile([128, mm_f], F32, tag="ps")
                    for m in range(mm_per_tile):
                        nc.tensor.matmul(
                            out=ps[32 * m:32 * m + 32, :],
                            lhsT=w_sb[:],
                            rhs=exp_res[t][:, m * mm_f:(m + 1) * mm_f],
                            start=True, stop=True,
                            tile_position=(0, 32 * m))
                    ln_scr = scratch.tile([128, mm_f], BF16, tag="ln_scr")
                    nc.scalar.activation(ln_scr[0:bank_rows, :],
                                         ps[0:bank_rows, :], Act.Ln,
                                         accum_out=s3_cols[:, t:t + 1])
                else:
                    for m in range(mm_per_tile):
                        ps = psum.tile([128, mm_f], F32, tag="ps")
                        nc.tensor.matmul(
                            out=ps[0:g, :], lhsT=w_sb[0:p, 0:g],
                            rhs=exp_res[t][:, m * mm_f:(m + 1) * mm_f],
                            start=True, stop=True)
                        ln_scr = scratch.tile([128, mm_f], F32, tag="ln_scr")
                        nc.scalar.activation(ln_scr[0:g, :], ps[0:g, :],
                                             Act.Ln,
                                             accum_out=s3_rem[:, m:m + 1])

            # write back (regions written above only)
            nc.sync.dma_start(s1c_d[:, 0:n_full], s1_cols[:, 0:n_full])
            if rem_g:
                nc.sync.dma_start(
                    s1c_d[0:rem_g * c, n_full:n_full + 1],
                    s1_cols[0:rem_g * c, n_full:n_full + 1])
                nc.sync.dma_start(s3r_d[:], s3_rem[:])
            if n_full:
                nc.sync.dma_start(s3c_d[:], s3_cols[:])

    nc.compile()

    meta = dict(n_cores=n_cores, c=c, npix=npix, tile_f=tile_f,
                n_full=n_full, rem_g=rem_g, g_full=g_full,
                mm_per_tile=mm_per_tile)
    return nc, meta


def host_layout(arr_cn, c, tile_f):
    """[c, npix] -> [n_chunks*c, tile_f], row (chunk*c + class)."""
    n_chunks = arr_cn.shape[1] // tile_f
    return np.ascontiguousarray(
        arr_cn.reshape(c, n_chunks, tile_f).transpose(1, 0, 2)
    ).reshape(n_chunks * c, tile_f)


_CACHE = {}


def _get_program():
    if "full" not in _CACHE:
        _CACHE["full"] = build()
    return _CACHE["full"]


def run_sharded(pred, target, trace=False, **spmd_kwargs):
    """pred/target: [B, C, H, W] float32. Returns (np.float32 scalar, results)."""
    pred = np.asarray(pred, dtype=np.float32)
    target = np.asarray(target, dtype=np.float32)
    b, c, h, w = pred.shape
    assert (b, c, h, w) == (B, C, H, W), (pred.shape,)

    nc, meta = _get_program()
    in_maps = [
        {"pred": host_layout(pred[i].reshape(c, h * w), c, TILE_F),
         "tgt": host_layout(target[i].reshape(c, h * w), c, TILE_F)}
        for i in range(N_CORES)
    ]
    res = run_bass_kernel_spmd(nc, in_maps, core_ids=list(range(N_CORES)),
                               trace=trace, **spmd_kwargs)
    out = finalize(res.results, b * h * w, meta)
    return out, res


def finalize(results, n_total, meta):
    """Combine per-core partials; exclude pad/garbage regions."""
    n_full, rem_g = meta["n_full"], meta["rem_g"]
    c, g_full = meta["c"], meta["g_full"]

    def _sum(r):
        s1c = r["s1c"].astype(np.float64)
        s1 = s1c[:, :n_full].sum()
        if rem_g:
            s1 += s1c[:rem_g * c, n_full].sum()
        s3 = 0.0
        if n_full:
            s3c = r["s3c"].astype(np.float64)  # [128, n_full]
            rows = s3c.reshape(-1, 32, s3c.shape[1])[:, :g_full, :]
            s3 += rows.sum()
        if rem_g:
            s3 += r["s3r"].astype(np.float64).sum()
        return s1, s3

    parts = [_sum(r) for r in results]
    s1 = sum(p[0] for p in parts)
    s3 = sum(p[1] for p in parts)
    cf = results[0]["cfg"].astype(np.float64).ravel()
    s2 = float(np.sum(np.where(cf > 0, cf * np.log(np.maximum(cf, 1e-30)), 0.0)))
    val = -(s1 + s2 - s3) / float(n_total)
    return np.array(val, dtype=np.float32)


def kernel(pred, target):
    out, _ = run_sharded(pred, target)
    return out


# revision 27
# speedup vs baseline: 1.0260x; 1.0260x over previous
"""Trainium2 Bass kernel for nn_BSLSegmenterV0 (histogram-binning weighted CE).

Math (target is exactly one-hot over the class axis C):
    cf[c]  = sum_n target[n, c]                      (global class histogram)
    S1     = sum_{n,c} target[n,c] * pred[n,c]
    S2     = sum_c cf[c] * ln(cf[c])
    S3     = sum_n ln( sum_c exp(pred[n,c]) * cf[c] )
    out    = -(S1 + S2 - S3) / N

Sharding: batch-parallel over 8 NeuronCores (one batch image each). The only
cross-core dependency is the 126-float cf partial histogram (AllGather +
on-chip fold); S1/S3 partials are returned per core and combined on the host.

Per-core dataflow (classes on partitions, pixels on the free axis; the host
pre-arranges each shard to [n_chunks*C, tile_f] chunk-major so every device
DMA is a contiguous 2-D block):
  pass A (streamed, DMA-bound): exp(pred) kept resident in SBUF as bf16;
      cf partials via ScalarE activation accum_out; S1 via VectorE mul+reduce.
  cf -> AllGather -> global cf -> block-diag bf16 W. This chain runs entirely
      on ScalarE + DMA so it never queues behind VectorE's pass-A backlog.
  pass B (from resident SBUF): per full tile, 4 col-tiled PE matmuls
      A = W^T @ exp(pred) fill one PSUM bank [128, 512] (rows 32j..32j+5 are
      real groups, rows 32j+6..32j+31 are forced to 1.0 via a ones-row in the
      moving tile and ones-columns in W, so ln() of the whole bank is safe);
      one ScalarE Ln activation with accum_out per tile yields sum ln(A).
"""

import os
import sys

for _p in ("/opt/trn_rl_repo", "/root/.axon_site/_ro/trn_rl_repo"):
    if os.path.isdir(_p) and _p not in sys.path:
        sys.path.append(_p)

import ml_dtypes
import numpy as np

import concourse.bacc as bacc
import concourse.bass as bass
import concourse.mybir as mybir
import concourse.tile as tile
from concourse.bass_utils import run_bass_kernel_spmd

F32 = mybir.dt.float32
BF16 = mybir.dt.bfloat16
Act = mybir.ActivationFunctionType

# full-problem config
B, C, H, W = 8, 21, 512, 512
N_CORES = 8
NPIX = H * W              # pixels per core (one batch image per core)
TILE_F = 2048             # pixels per chunk (free-dim of a stream tile)
MM_F = 512                # matmul moving free dim (one PSUM bank of fp32)


def build(n_cores=N_CORES, c=C, npix=NPIX, tile_f=TILE_F, mm_f=MM_F):
    """Build the SPMD Bass program. Returns (nc, meta)."""
    assert npix % tile_f == 0
    n_chunks = npix // tile_f
    g_full = 128 // c                      # class-groups stacked on partitions
    n_full = n_chunks // g_full            # full tiles
    rem_g = n_chunks % g_full              # groups in the remainder tile
    tiles = [g_full] * n_full + ([rem_g] if rem_g else [])
    nt = len(tiles)
    mm_per_tile = tile_f // mm_f
    assert mm_per_tile <= 4, "col-tiling uses one PSUM bank of 4 x 32 rows"
    pmax = g_full * c                      # 126

    nc = bacc.Bacc("TRN2", target_bir_lowering=False, debug=False,
                   num_devices=n_cores)

    # host pre-arranged layout: row (chunk*c + class), contiguous rows
    pred_d = nc.dram_tensor("pred", [n_chunks * c, tile_f], F32,
                            kind="ExternalInput").ap()
    tgt_d = nc.dram_tensor("tgt", [n_chunks * c, tile_f], F32,
                           kind="ExternalInput").ap()
    s1c_d = nc.dram_tensor("s1c", [pmax, nt], F32, kind="ExternalOutput").ap()
    bank_rows = 32 * mm_per_tile
    s3c_d = nc.dram_tensor("s3c", [bank_rows, max(n_full, 1)], F32,
                           kind="ExternalOutput").ap()
    s3r_d = nc.dram_tensor("s3r", [max(rem_g, 1), mm_per_tile], F32,
                           kind="ExternalOutput").ap()
    cfg_d = nc.dram_tensor("cfg", [c, 1], F32, kind="ExternalOutput").ap()

    cc_space = "Shared" if n_cores > 4 else "Local"
    cc_in = nc.dram_tensor("cc_in", [pmax], F32)
    cc_out = nc.dram_tensor("cc_out", [n_cores * pmax], F32,
                            addr_space=cc_space)
    dum_in = nc.dram_tensor("dum_in", [32], F32)
    dum_out = nc.dram_tensor("dum_out", [n_cores * 32], F32,
                             addr_space=cc_space)
    ones_d = nc.inline_tensor(
        np.ones((1, tile_f), dtype=ml_dtypes.bfloat16), name="ones_bf16")

    with tile.TileContext(nc) as tc:
        with (
            tc.tile_pool(name="tstreams", bufs=5) as tstreams,
            tc.tile_pool(name="pstreams", bufs=3) as pstreams,
            tc.tile_pool(name="tring", bufs=6) as tring,
            tc.tile_pool(name="scratch", bufs=2) as scratch,
            tc.tile_pool(name="resident", bufs=1) as resident,
            tc.tile_pool(name="stats", bufs=1) as stats,
            tc.tile_pool(name="psum", bufs=6, space="PSUM") as psum,
        ):
            cf_cols = stats.tile([pmax, nt], F32, tag="cf_cols")
            s1_cols = stats.tile([pmax, nt], F32, tag="s1_cols")
            s3_cols = stats.tile([bank_rows, max(n_full, 1)], F32, tag="s3_cols")
            s3_rem = stats.tile([max(rem_g, 1), mm_per_tile], F32, tag="s3_rem")
            # W: [127, 32] bf16; cols 0..g_full-1 block-diag cf, row 126 has
            # 1.0 in cols g_full..31 (pairs with the ones-row of moving tiles
            # so col-tiled PSUM pad rows become ln(1.0))
            w_sb = stats.tile([pmax + 1, 32], BF16, tag="w_sb")
            nc.scalar.memzero(w_sb[:])
            # warm up the ncfw collective path early (overlaps pass A)
            dum_sb = stats.tile([32, 1], F32, tag="dum_sb")
            nc.vector.memset(dum_sb[:], 0.0)
            nc.sync.dma_start(dum_in[:], dum_sb[:, 0])
            nc.gpsimd.collective_compute(
                "AllGather", mybir.AluOpType.bypass,
                replica_groups=[list(range(n_cores))],
                ins=[dum_in[:]], outs=[dum_out[:]])
            nc.sync.dma_start(w_sb[pmax:pmax + 1, g_full:32],
                              ones_d[0:1, 0:32 - g_full])
            if rem_g:
                # remainder cf column fills only rem_g*c rows; pre-zero the
                # whole column (engine ops need 32-aligned start partitions,
                # so we can't zero just the tail)
                nc.scalar.memzero(cf_cols[:, n_full:n_full + 1])

            # ---- pass A: stream target+pred, build resident exp(pred) ----
            # tgt DMAs are issued LEAD tiles ahead of pred DMAs so the class
            # histogram (collective input) completes well before pred streaming
            # does, hiding the AllGather under the pass-A tail.
            LEAD = 5
            t_tiles = []

            def issue_tgt(t, g):
                p = g * c
                r0 = t * g_full * c
                t_tile = tstreams.tile([p, tile_f], F32, tag="t_stream")
                nc.sync.dma_start(t_tile[:], tgt_d[r0:r0 + p, :])
                t_tiles.append(t_tile)

            for t in range(min(LEAD, nt)):
                issue_tgt(t, tiles[t])
            exp_res = []
            for t, g in enumerate(tiles):
                p = g * c
                r0 = t * g_full * c
                p_tile = pstreams.tile([p, tile_f], F32, tag="p_stream")
                nc.sync.dma_start(p_tile[:], pred_d[r0:r0 + p, :])
                if t + LEAD < nt:
                    issue_tgt(t + LEAD, tiles[t + LEAD])
                t_tile = t_tiles[t]

                full = g == g_full
                e_tile = resident.tile([p + (1 if full else 0), tile_f], BF16,
                                       tag=f"exp{t}")
                exp_res.append(e_tile)
                if full:  # ones-row pairs with W's ones-columns in pass B
                    nc.gpsimd.dma_start(e_tile[p:p + 1, :], ones_d[0:1, :])
                nc.scalar.activation(e_tile[0:p, :], p_tile[:], Act.Exp)
                # cf partial: ScalarE identity with accumulate output
                a_scr = scratch.tile([p, tile_f], mybir.dt.float8e4, tag="a_scr")
                nc.scalar.activation(a_scr[:], t_tile[:], Act.Identity,
                                     accum_out=cf_cols[0:p, t:t + 1])
                # Cast tgt to a bf16 ring so the fp32 tgt slot frees without
                # waiting for pred (decouples the two DMA streams; cf must
                # complete well before pred streaming does).
                t_bf = tring.tile([p, tile_f], BF16, tag="t_bf")
                nc.vector.tensor_copy(t_bf[:], t_tile[:])
                # S1 partial: (tgt*pred) then free-axis reduce on VectorE
                v_scr = scratch.tile([p, tile_f], BF16, tag="v_scr")
                nc.vector.tensor_mul(v_scr[:], t_bf[:], p_tile[:])
                nc.vector.tensor_reduce(s1_cols[0:p, t:t + 1], v_scr[:],
                                        axis=mybir.AxisListType.X,
                                        op=mybir.AluOpType.add)

            # ---- cf: ScalarE pre-fold -> AllGather [126] -> fold -> W ----
            cf_part = stats.tile([pmax, 1], F32, tag="cf_part")
            f_scr = stats.tile([pmax, nt], BF16, tag="f_scr")
            nc.scalar.activation(f_scr[:], cf_cols[:], Act.Identity,
                                 accum_out=cf_part[:])
            nc.sync.dma_start(cc_in[:], cf_part[:, 0])
            nc.gpsimd.collective_compute(
                "AllGather", mybir.AluOpType.bypass,
                replica_groups=[list(range(n_cores))],
                ins=[cc_in[:]], outs=[cc_out[:]])
            # dram element (r, j, ch) -> sbuf [ch, (r j)]
            ncols = n_cores * g_full
            cf_all = stats.tile([c, ncols], F32, tag="cf_all")
            nc.sync.dma_start(
                cf_all[:].rearrange("ch (r j) -> ch r j", r=n_cores),
                cc_out.rearrange("(r j ch) -> ch r j", ch=c, j=g_full))
            cf_g = stats.tile([c, 1], F32, tag="cf_g")
            g_scr = stats.tile([c, ncols], BF16, tag="g_scr")
            nc.scalar.activation(g_scr[:], cf_all[:], Act.Identity,
                                 accum_out=cf_g[:])
            nc.sync.dma_start(cfg_d[:], cf_g[:])
            cf_gb = stats.tile([c, 1], BF16, tag="cf_gb")
            nc.scalar.activation(cf_gb[:], cf_g[:], Act.Copy)
            for j in range(g_full):
                nc.sync.dma_start(w_sb[j * c:(j + 1) * c, j:j + 1], cf_gb[:])

            # ---- pass B: A = W^T @ exp(pred); S3 += sum ln(A) ----
            for t, g in enumerate(tiles):
                p = g * c
                if g == g_full:
                    ps = psum.tile([128, mm_f], F32, tag="ps")
                    for m in range(mm_per_tile):
                        nc.tensor.matmul(
                            out=ps[32 * m:32 * m + 32, :],
                            lhsT=w_sb[:],
                            rhs=exp_res[t][:, m * mm_f:(m + 1) * mm_f],
                            start=True, stop=True,
                            tile_position=(0, 32 * m))
                    ln_scr = scratch.tile([128, mm_f], BF16, tag="ln_scr")
                    nc.scalar.activation(ln_scr[0:bank_rows, :],
                                         ps[0:bank_rows, :], Act.Ln,
                                         accum_out=s3_cols[:, t:t + 1])
                else:
                    for m in range(mm_per_tile):
                        ps = psum.tile([128, mm_f], F32, tag="ps")
                        nc.tensor.matmul(
                            out=ps[0:g, :], lhsT=w_sb[0:p, 0:g],
                            rhs=exp_res[t][:, m * mm_f:(m + 1) * mm_f],
                            start=True, stop=True)
                        ln_scr = scratch.tile([128, mm_f], F32, tag="ln_scr")
                        nc.scalar.activation(ln_scr[0:g, :], ps[0:g, :],
                                             Act.Ln,
                                             accum_out=s3_rem[:, m:m + 1])

            # write back (regions written above only)
            nc.sync.dma_start(s1c_d[:, 0:n_full], s1_cols[:, 0:n_full])
            if rem_g:
                nc.sync.dma_start(
                    s1c_d[0:rem_g * c, n_full:n_full + 1],
                    s1_cols[0:rem_g * c, n_full:n_full + 1])
                nc.sync.dma_start(s3r_d[:], s3_rem[:])
            if n_full:
                nc.sync.dma_start(s3c_d[:], s3_cols[:])

    nc.compile()

    meta = dict(n_cores=n_cores, c=c, npix=npix, tile_f=tile_f,
                n_full=n_full, rem_g=rem_g, g_full=g_full,
                mm_per_tile=mm_per_tile)
    return nc, meta


def host_layout(arr_cn, c, tile_f):
    """[c, npix] -> [n_chunks*c, tile_f], row (chunk*c + class)."""
    n_chunks = arr_cn.shape[1] // tile_f
    return np.ascontiguousarray(
        arr_cn.reshape(c, n_chunks, tile_f).transpose(1, 0, 2)
    ).reshape(n_chunks * c, tile_f)


_CACHE = {}


def _get_program():
    if "full" not in _CACHE:
        _CACHE["full"] = build()
    return _CACHE["full"]


def run_sharded(pred, target, trace=False, **spmd_kwargs):
    """pred/target: [B, C, H, W] float32. Returns (np.float32 scalar, results)."""
    pred = np.asarray(pred, dtype=np.float32)
    target = np.asarray(target, dtype=np.float32)
    b, c, h, w = pred.shape
    assert (b, c, h, w) == (B, C, H, W), (pred.shape,)

    nc, meta = _get_program()
    in_maps = [
        {"pred": host_layout(pred[i].reshape(c, h * w), c, TILE_F),
         "tgt": host_layout(target[i].reshape(c, h * w), c, TILE_F)}
        for i in range(N_CORES)
    ]
    res = run_bass_kernel_spmd(nc, in_maps, core_ids=list(range(N_CORES)),
                               trace=trace, **spmd_kwargs)
    out = finalize(res.results, b * h * w, meta)
    return out, res


def finalize(results, n_total, meta):
    """Combine per-core partials; exclude pad/garbage regions."""
    n_full, rem_g = meta["n_full"], meta["rem_g"]
    c, g_full = meta["c"], meta["g_full"]

    def _sum(r):
        s1c = r["s1c"].astype(np.float64)
        s1 = s1c[:, :n_full].sum()
        if rem_g:
            s1 += s1c[:rem_g * c, n_full].sum()
        s3 = 0.0
        if n_full:
            s3c = r["s3c"].astype(np.float64)  # [128, n_full]
            rows = s3c.reshape(-1, 32, s3c.shape[1])[:, :g_full, :]
            s3 += rows.sum()
        if rem_g:
            s3 += r["s3r"].astype(np.float64).sum()
        return s1, s3

    parts = [_sum(r) for r in results]
    s1 = sum(p[0] for p in parts)
    s3 = sum(p[1] for p in parts)
    cf = results[0]["cfg"].astype(np.float64).ravel()
    s2 = float(np.sum(np.where(cf > 0, cf * np.log(np.maximum(cf, 1e-30)), 0.0)))
    val = -(s1 + s2 - s3) / float(n_total)
    return np.array(val, dtype=np.float32)


def kernel(pred, target):
    out, _ = run_sharded(pred, target)
    return out
